# revision 1
# baseline (speedup 1.0000x reference)
"""Trainium2 Bass kernel for the sparse-attention scores module.

Computes, for each batch b:
    scores[b, :] = softmax_s( v . tanh(W1 @ static[b] + W2 @ dynamic[b] + W3 @ hidden[b]) )
with W = [W1 | W2 | W3] of shape [H, 3H], static/dynamic [B, H, S], hidden [B, H].

Sharding: data-parallel over B across 8 NeuronCores (8 batches per core).
Per core the kernel streams 64 MiB of encoder data from HBM (memory-bound),
runs the two [256,256]@[256,4096] matmuls per batch in fp32r on the PE,
adds the per-batch bias and applies tanh on the ACT engine, reduces with v
via a second PE matmul, and finishes with an exp/normalize softmax epilogue.
"""

import sys

sys.path.insert(0, "/opt/trn_rl_repo")

import numpy as np

B, H, S = 64, 256, 4096
N_CORES = 8
BPC = B // N_CORES          # batches per core
KK = H // 128               # 2 contraction chunks
MM = H // 128               # 2 output-row chunks
NCH = S // 512              # 8 psum column chunks
NQ = 4                      # input DMA quarters along s
SQ = S // NQ                # 1024 columns per quarter


def build_bass(reps: int = 1, loop_iters: int = 0):
    """Build the per-core Bass program. reps>1 unrolls the whole computation
    multiple times; loop_iters>0 additionally wraps the unrolled body in a
    hardware loop. Both are used only for timing by differencing."""
    import contextlib

    import concourse.bacc as bacc
    import concourse.tile as tile
    from concourse import mybir

    f32 = mybir.dt.float32
    f32r = mybir.dt.float32r

    nc = bacc.Bacc(None)

    xs = nc.dram_tensor("xs", [BPC, H, S], f32r, kind="ExternalInput")
    xd = nc.dram_tensor("xd", [BPC, H, S], f32r, kind="ExternalInput")
    wt = nc.dram_tensor("wt", [128, 12, 128], f32r, kind="ExternalInput")
    ht = nc.dram_tensor("ht", [128, KK, BPC], f32r, kind="ExternalInput")
    vt = nc.dram_tensor("vt", [128, KK], f32r, kind="ExternalInput")
    out = nc.dram_tensor("out", [BPC, S], f32, kind="ExternalOutput")

    with tile.TileContext(nc) as tc:
        with (
            tc.tile_pool(name="consts", bufs=1) as consts,
            tc.tile_pool(name="xpool", bufs=2) as xpool,
            tc.tile_pool(name="tpool", bufs=6) as tpool,
            tc.tile_pool(name="spool", bufs=2) as spool,
            tc.tile_pool(name="mpsum", bufs=4, space="PSUM") as mpsum,
            tc.tile_pool(name="vpsum", bufs=2, space="PSUM") as vpsum,
            tc.tile_pool(name="spsum", bufs=2, space="PSUM") as spsum,
        ):
            wt_sb = consts.tile([128, 12, 128], f32r)
            nc.sync.dma_start(out=wt_sb, in_=wt[:, :, :])
            ht_sb = consts.tile([128, KK, BPC], f32r)
            nc.sync.dma_start(out=ht_sb, in_=ht[:, :, :])
            vt_sb = consts.tile([128, KK], f32r)
            nc.sync.dma_start(out=vt_sb, in_=vt[:, :])

            # Inline 0/1 masks for the softmax normalization matmuls:
            # bsum[b] = sum_n esums[8b+n]; brep[8b+n] = bsum[b].
            ma_np = np.zeros((64, BPC), np.float32)
            mb_np = np.zeros((BPC, 64), np.float32)
            for p in range(64):
                ma_np[p, p // NCH] = 1.0
                mb_np[p // NCH, p] = 1.0
            ma_dram = nc.inline_tensor(ma_np, name="ma")
            mb_dram = nc.inline_tensor(mb_np, name="mb")
            ma_sb = consts.tile([64, BPC], f32)
            nc.sync.dma_start(out=ma_sb, in_=ma_dram[:, :])
            mb_sb = consts.tile([BPC, 64], f32)
            nc.sync.dma_start(out=mb_sb, in_=mb_dram[:, :])

            # Per-batch bias: bias[m*128+h', b] = (W3 @ hidden[b])[m*128+h']
            bias_sb = consts.tile([128, MM, BPC], f32)
            for m in range(MM):
                bias_ps = spsum.tile([128, BPC], f32, tag="small")
                for kk in range(KK):
                    nc.tensor.matmul(
                        bias_ps,
                        lhsT=wt_sb[:, 8 + kk * 2 + m, :],
                        rhs=ht_sb[:, kk, :],
                        start=(kk == 0),
                        stop=(kk == KK - 1),
                    )
                nc.vector.tensor_copy(out=bias_sb[:, m, :], in_=bias_ps)

            loop_cm = (
                tc.For_i(0, loop_iters, 1) if loop_iters else contextlib.nullcontext()
            )
            with loop_cm:
              for _ in range(reps):
                # Scores live as [64, 512] with partition p = 8*b + n so the
                # epilogue runs on all 64 partitions at once.
                scores64 = spool.tile([64, 512], f32, tag="scores")
                pending = None
                for b in range(BPC):
                    # Stream the two encoder tensors in 1 MiB quarters so the
                    # PE can start each pair of column chunks as soon as its
                    # slice of data lands.
                    xq = []
                    for q in range(NQ):
                        pair = []
                        for t, dram in ((0, xs), (1, xd)):
                            xt = xpool.tile([128, KK, SQ], f32r, tag=f"x{t}{q}")
                            src = dram[b, :, q * SQ : (q + 1) * SQ].rearrange(
                                "(kk p) s -> p kk s", p=128
                            )
                            nc.sync.dma_start(out=xt, in_=src)
                            pair.append(xt)
                        xq.append(pair)

                    def emit_vdot(pend):
                        # v-dot runs one chunk late so the tanh results are
                        # ready and the PE never waits on the ACT engine.
                        row, vp, tts = pend
                        for m in range(MM):
                            nc.tensor.matmul(
                                vp,
                                lhsT=vt_sb[:, m : m + 1],
                                rhs=tts[m],
                                start=(m == 0),
                                stop=(m == MM - 1),
                            )
                        # Compute engines may only address partition bases
                        # that are multiples of 32, so the chunk is drained to
                        # partition 0 and a tiny SBUF->SBUF DMA places it at
                        # partition 8b+n of the scores tile.
                        stage = tpool.tile([1, 512], f32, tag="stage")
                        nc.vector.tensor_copy(out=stage, in_=vp)
                        nc.gpsimd.dma_start(
                            out=scores64[row : row + 1, :],
                            in_=stage,
                        )

                    for n in range(NCH):
                        q, r = divmod(n, NCH // NQ)
                        tts = []
                        for m in range(MM):
                            ps = mpsum.tile([128, 512], f32, tag="ps")
                            i = 0
                            for t in range(2):
                                for kk in range(KK):
                                    nc.tensor.matmul(
                                        ps,
                                        lhsT=wt_sb[:, t * 4 + kk * 2 + m, :],
                                        rhs=xq[q][t][:, kk, r * 512 : (r + 1) * 512],
                                        start=(i == 0),
                                        stop=(i == 3),
                                    )
                                    i += 1
                            tt = tpool.tile([128, 512], f32r, tag="tt")
                            nc.scalar.activation(
                                out=tt,
                                in_=ps,
                                func=mybir.ActivationFunctionType.Tanh,
                                bias=bias_sb[:, m, b : b + 1],
                                scale=1.0,
                            )
                            tts.append(tt)
                        if pending is not None:
                            emit_vdot(pending)
                        vp = vpsum.tile([1, 512], f32, tag="vp")
                        pending = (b * NCH + n, vp, tts)
                # flush the last batch's final v-dot after the loop
                emit_vdot(pending)

                # Softmax epilogue. Scores are small (|s| < ~6), so skip the
                # max subtraction: softmax = exp(s) / sum(exp(s)). The
                # per-batch sums are formed from the per-partition accum via
                # two tiny 0/1-mask matmuls (sum over n, then broadcast).
                esums = spool.tile([64, 1], f32, tag="esums")
                nc.scalar.activation(
                    out=scores64,
                    in_=scores64,
                    func=mybir.ActivationFunctionType.Exp,
                    accum_out=esums,
                )
                bsum_ps = spsum.tile([BPC, 1], f32, tag="small")
                nc.tensor.matmul(bsum_ps, lhsT=ma_sb, rhs=esums,
                                 start=True, stop=True)
                bsum_sb = spool.tile([BPC, 1], f32, tag="bsum")
                nc.vector.tensor_copy(out=bsum_sb, in_=bsum_ps)
                brep_ps = spsum.tile([64, 1], f32, tag="small")
                nc.tensor.matmul(brep_ps, lhsT=mb_sb, rhs=bsum_sb,
                                 start=True, stop=True)
                recip = spool.tile([64, 1], f32, tag="recip")
                nc.vector.reciprocal(out=recip, in_=brep_ps)
                nc.vector.tensor_scalar_mul(out=scores64, in0=scores64, scalar1=recip)
                nc.gpsimd.dma_start(
                    out=out[:, :].rearrange("b (n s) -> (b n) s", n=NCH),
                    in_=scores64,
                )

    nc.finalize()
    return nc


def prep_shared_inputs(W: np.ndarray, v: np.ndarray, decoder_hidden: np.ndarray):
    """Host-side layout marshaling of the small replicated parameters."""
    W = np.ascontiguousarray(W, dtype=np.float32)
    wt_tiles = np.empty((128, 12, 128), np.float32)
    for t in range(3):
        Wt = W[:, t * H : (t + 1) * H].T  # [k, h]
        for kk in range(KK):
            for m in range(MM):
                j = t * 4 + kk * 2 + m
                wt_tiles[:, j, :] = Wt[kk * 128 : (kk + 1) * 128, m * 128 : (m + 1) * 128]
    vt = np.ascontiguousarray(v[0].reshape(KK, 128).T, dtype=np.float32)  # [p, kk]
    hT = decoder_hidden[0].T.astype(np.float32)  # [H, B]
    return wt_tiles, vt, hT


_CACHED = {}


def _get_nc(reps: int = 1, loop_iters: int = 0):
    key = (reps, loop_iters)
    if key not in _CACHED:
        _CACHED[key] = build_bass(reps, loop_iters)
    return _CACHED[key]


def make_in_maps(static_enc, dynamic_enc, decoder_hidden, W, v):
    wt_tiles, vt, hT = prep_shared_inputs(W, v, decoder_hidden)
    static_enc = np.ascontiguousarray(static_enc, dtype=np.float32)
    dynamic_enc = np.ascontiguousarray(dynamic_enc, dtype=np.float32)
    in_maps = []
    for c in range(N_CORES):
        b0 = c * BPC
        ht_c = np.ascontiguousarray(
            hT[:, b0 : b0 + BPC].reshape(KK, 128, BPC).transpose(1, 0, 2)
        )  # [p, kk, b]
        in_maps.append(
            {
                "xs": static_enc[b0 : b0 + BPC],
                "xd": dynamic_enc[b0 : b0 + BPC],
                "wt": wt_tiles,
                "ht": ht_c,
                "vt": vt,
            }
        )
    return in_maps


def kernel(static_enc, dynamic_enc, decoder_hidden, W, v):
    from concourse.bass_utils import run_bass_kernel_spmd

    nc = _get_nc(reps=1)
    in_maps = make_in_maps(static_enc, dynamic_enc, decoder_hidden, W, v)
    res = run_bass_kernel_spmd(nc, in_maps, core_ids=list(range(N_CORES)))
    return np.concatenate([r["out"] for r in res.results], axis=0)



# revision 3
# speedup vs baseline: 1.1612x; 1.1612x over previous
"""Trainium2 Bass kernel for the sparse-attention scores module.

Computes, for each batch b:
    scores[b, :] = softmax_s( v . tanh(W1 @ static[b] + W2 @ dynamic[b] + W3 @ hidden[b]) )
with W = [W1 | W2 | W3] of shape [H, 3H], static/dynamic [B, H, S], hidden [B, H].

Sharding: data-parallel over B across 8 NeuronCores (8 batches per core).
Per core the kernel streams 64 MiB of encoder data from HBM (memory-bound),
runs the two [256,256]@[256,4096] matmuls per batch in fp32r on the PE,
adds the per-batch bias and applies tanh on the ACT engine, and reduces with
v via a masked-lhsT PE matmul whose [8,512] PSUM output lands each batch's
scores directly on partition b.  A DVE add merges chunks into a [8,4096]
scores tile, so no SBUF->SBUF scatter DMAs are needed and the softmax
epilogue is a per-partition exp/normalize.
"""

import sys

sys.path.insert(0, "/opt/trn_rl_repo")

import numpy as np

B, H, S = 64, 256, 4096
N_CORES = 8
BPC = B // N_CORES          # batches per core
KK = H // 128               # 2 contraction chunks
MM = H // 128               # 2 output-row chunks
NCH = S // 512              # 8 psum column chunks
NQ = 4                      # input DMA quarters along s
SQ = S // NQ                # 1024 columns per quarter


def build_bass(reps: int = 1, loop_iters: int = 0):
    """Build the per-core Bass program. reps>1 unrolls the whole computation
    multiple times; loop_iters>0 additionally wraps the unrolled body in a
    hardware loop. Both are used only for timing by differencing."""
    import contextlib

    import concourse.bacc as bacc
    import concourse.tile as tile
    from concourse import mybir

    f32 = mybir.dt.float32
    f32r = mybir.dt.float32r

    nc = bacc.Bacc(None)

    xc = nc.dram_tensor("xc", [BPC, 2, H, S], f32r, kind="ExternalInput")
    wt = nc.dram_tensor("wt", [128, 12, 128], f32r, kind="ExternalInput")
    ht = nc.dram_tensor("ht", [128, KK, BPC], f32r, kind="ExternalInput")
    vm = nc.dram_tensor("vm", [128, MM, BPC, BPC], f32r, kind="ExternalInput")
    out = nc.dram_tensor("out", [BPC, S], f32, kind="ExternalOutput")

    with tile.TileContext(nc) as tc:
        with (
            tc.tile_pool(name="consts", bufs=1) as consts,
            tc.tile_pool(name="xpool", bufs=6) as xpool,
            tc.tile_pool(name="tpool", bufs=6) as tpool,
            tc.tile_pool(name="spool", bufs=2) as spool,
            tc.tile_pool(name="mpsum", bufs=4, space="PSUM") as mpsum,
            tc.tile_pool(name="vpsum", bufs=2, space="PSUM") as vpsum,
            tc.tile_pool(name="spsum", bufs=2, space="PSUM") as spsum,
        ):
            wt_sb = consts.tile([128, 12, 128], f32r)
            nc.sync.dma_start(out=wt_sb, in_=wt[:, :, :])
            ht_sb = consts.tile([128, KK, BPC], f32r)
            nc.sync.dma_start(out=ht_sb, in_=ht[:, :, :])
            vm_sb = consts.tile([128, MM, BPC, BPC], f32r)
            nc.sync.dma_start(out=vm_sb, in_=vm[:, :, :, :])

            # Per-batch bias: bias[m*128+h', b] = (W3 @ hidden[b])[m*128+h']
            bias_sb = consts.tile([128, MM, BPC], f32)
            for m in range(MM):
                bias_ps = spsum.tile([128, BPC], f32, tag="small")
                for kk in range(KK):
                    nc.tensor.matmul(
                        bias_ps,
                        lhsT=wt_sb[:, 8 + kk * 2 + m, :],
                        rhs=ht_sb[:, kk, :],
                        start=(kk == 0),
                        stop=(kk == KK - 1),
                    )
                nc.vector.tensor_copy(out=bias_sb[:, m, :], in_=bias_ps)

            loop_cm = (
                tc.For_i(0, loop_iters, 1) if loop_iters else contextlib.nullcontext()
            )
            with loop_cm:
              for _ in range(reps):
                # Scores live as [8, 4096] with partition p = batch b, so the
                # epilogue is a plain per-partition exp/normalize and the
                # output DMA is contiguous.
                scores8 = spool.tile([BPC, S], f32, tag="scores")
                esums = spool.tile([BPC, NCH], f32, tag="esums")
                pending = None

                def emit_vdot(pend):
                    # v-dot runs one chunk late so the tanh results are
                    # ready and the PE never waits on the ACT engine.  The
                    # lhsT is v masked into column b, so the [8,512] PSUM
                    # result lands on partition b directly (rows != b are
                    # exact zeros) and a DVE add/copy merges it into the
                    # scores tile -- no scatter DMA.
                    b, n, vp, tts = pend
                    for m in range(MM):
                        nc.tensor.matmul(
                            vp,
                            lhsT=vm_sb[:, m, b, :],
                            rhs=tts[m],
                            start=(m == 0),
                            stop=(m == MM - 1),
                        )
                    sl = scores8[:, n * 512 : (n + 1) * 512]
                    if b == 0:
                        nc.vector.tensor_copy(out=sl, in_=vp)
                    else:
                        nc.vector.tensor_add(out=sl, in0=vp, in1=sl)
                    if b == BPC - 1:
                        # Last batch for this column chunk: exponentiate in
                        # place while later chunks still stream.  Scores are
                        # small (|s| < ~6), so skip the max subtraction.
                        nc.scalar.activation(
                            out=sl,
                            in_=sl,
                            func=mybir.ActivationFunctionType.Exp,
                            accum_out=esums[:, n : n + 1],
                        )

                for b in range(BPC):
                    # Stream both encoder tensors in 2 MiB quarters so the
                    # PE can start each pair of column chunks as soon as its
                    # slice of data lands.
                    xq = []
                    for q in range(NQ):
                        xt = xpool.tile([128, 2, KK, SQ], f32r, tag="xq")
                        src = xc[b, :, :, q * SQ : (q + 1) * SQ].rearrange(
                            "t (kk p) s -> p t kk s", p=128
                        )
                        nc.sync.dma_start(out=xt, in_=src)
                        xq.append(xt)

                    for n in range(NCH):
                        q, r = divmod(n, NCH // NQ)
                        tts = []
                        for m in range(MM):
                            ps = mpsum.tile([128, 512], f32, tag="ps")
                            i = 0
                            for t in range(2):
                                for kk in range(KK):
                                    nc.tensor.matmul(
                                        ps,
                                        lhsT=wt_sb[:, t * 4 + kk * 2 + m, :],
                                        rhs=xq[q][:, t, kk, r * 512 : (r + 1) * 512],
                                        start=(i == 0),
                                        stop=(i == 3),
                                    )
                                    i += 1
                            tt = tpool.tile([128, 512], f32r, tag="tt")
                            nc.scalar.activation(
                                out=tt,
                                in_=ps,
                                func=mybir.ActivationFunctionType.Tanh,
                                bias=bias_sb[:, m, b : b + 1],
                                scale=1.0,
                            )
                            tts.append(tt)
                        if pending is not None:
                            emit_vdot(pending)
                        vp = vpsum.tile([BPC, 512], f32, tag="vp")
                        pending = (b, n, vp, tts)
                # flush the last batch's final v-dot (and its exp) after the loop
                emit_vdot(pending)
                pending = None

                # Softmax normalization: per-batch sum of the chunk exp-sums,
                # reciprocal, then scale.  The multiplies are split across
                # DVE / ACT / GPSIMD so the tail is short.
                bsum = spool.tile([BPC, 1], f32, tag="bsum")
                nc.vector.reduce_sum(out=bsum, in_=esums, axis=mybir.AxisListType.X)
                recip = spool.tile([BPC, 1], f32, tag="recip")
                nc.vector.reciprocal(out=recip, in_=bsum)
                nc.vector.tensor_scalar_mul(
                    out=scores8[:, 0:2048], in0=scores8[:, 0:2048], scalar1=recip
                )
                nc.scalar.activation(
                    out=scores8[:, 2048:3328],
                    in_=scores8[:, 2048:3328],
                    func=mybir.ActivationFunctionType.Copy,
                    scale=recip,
                )
                nc.gpsimd.tensor_scalar_mul(
                    out=scores8[:, 3328:4096], in0=scores8[:, 3328:4096], scalar1=recip
                )
                # Output DMA rides the (idle) gpsimd queue so its wait on the
                # epilogue never blocks the next rep's input stream on the
                # sync queue.
                nc.gpsimd.dma_start(out=out[:, :], in_=scores8)

    nc.finalize()
    return nc


def prep_shared_inputs(W: np.ndarray, v: np.ndarray, decoder_hidden: np.ndarray):
    """Host-side layout marshaling of the small replicated parameters."""
    W = np.ascontiguousarray(W, dtype=np.float32)
    wt_tiles = np.empty((128, 12, 128), np.float32)
    for t in range(3):
        Wt = W[:, t * H : (t + 1) * H].T  # [k, h]
        for kk in range(KK):
            for m in range(MM):
                j = t * 4 + kk * 2 + m
                wt_tiles[:, j, :] = Wt[kk * 128 : (kk + 1) * 128, m * 128 : (m + 1) * 128]
    vt = np.ascontiguousarray(v[0].reshape(KK, 128).T, dtype=np.float32)  # [p, m]
    vm_tiles = np.zeros((128, MM, BPC, BPC), np.float32)
    for b in range(BPC):
        vm_tiles[:, :, b, b] = vt
    hT = decoder_hidden[0].T.astype(np.float32)  # [H, B]
    return wt_tiles, vm_tiles, hT


_CACHED = {}


def _get_nc(reps: int = 1, loop_iters: int = 0):
    key = (reps, loop_iters)
    if key not in _CACHED:
        _CACHED[key] = build_bass(reps, loop_iters)
    return _CACHED[key]


def make_in_maps(static_enc, dynamic_enc, decoder_hidden, W, v):
    wt_tiles, vm_tiles, hT = prep_shared_inputs(W, v, decoder_hidden)
    static_enc = np.ascontiguousarray(static_enc, dtype=np.float32)
    dynamic_enc = np.ascontiguousarray(dynamic_enc, dtype=np.float32)
    in_maps = []
    for c in range(N_CORES):
        b0 = c * BPC
        ht_c = np.ascontiguousarray(
            hT[:, b0 : b0 + BPC].reshape(KK, 128, BPC).transpose(1, 0, 2)
        )  # [p, kk, b]
        xc_c = np.ascontiguousarray(
            np.stack(
                [static_enc[b0 : b0 + BPC], dynamic_enc[b0 : b0 + BPC]], axis=1
            )
        )  # [b, t, h, s]
        in_maps.append(
            {
                "xc": xc_c,
                "wt": wt_tiles,
                "ht": ht_c,
                "vm": vm_tiles,
            }
        )
    return in_maps


def kernel(static_enc, dynamic_enc, decoder_hidden, W, v):
    from concourse.bass_utils import run_bass_kernel_spmd

    nc = _get_nc(reps=1)
    in_maps = make_in_maps(static_enc, dynamic_enc, decoder_hidden, W, v)
    res = run_bass_kernel_spmd(nc, in_maps, core_ids=list(range(N_CORES)))
    return np.concatenate([r["out"] for r in res.results], axis=0)
